# revision 11
# baseline (speedup 1.0000x reference)
"""Trainium2 Bass kernel for nn_MLPLoraSubspace.

Math: A = sum(alphas_A * controls_A, 0)  (256,)
      Bv = sum(alphas_A.T * controls_B, 1)  (4096,)
      W = A outer Bv  (rank-1)  -> out = (x @ Bv) outer A + bias
      BatchNorm(training stats) then LeakyReLU(0.2).

Because W is rank-1, out[i,j] = A[j]*t[i] + bias[j] with t = x @ Bv.
Batch stats:  mean_j = A_j*mean(t) + bias_j,  var_j = A_j^2*var(t), so
  act[i,j] = lrelu( u_j * (t[i]-mean_t) + beta_j ),
  u_j = gamma_j*A_j/sqrt(A_j^2*var_t+eps).  Bias cancels exactly.

Fast path (beta==0 and gamma*A >= 0, true for the reference inputs):
  lrelu(u_j * z) = u_j * lrelu(z), so the activation collapses onto the
  tiny t column and phase 3 is a pure outer product.

Sharding: data-parallel over batch, 8 cores x 2048 rows. Per-core partial
(sum t, sum t^2) is AllGather'd (2 floats) to form global batch stats.

v2 layout: x streamed as 16x 2MB tile DMAs alternating the two HWDGE
queues (sync/scalar); per tile ONE DVE tensor_tensor_reduce does
mult+rowsum; consts arrive as one [1,4864] strip and are broadcast
across partitions with TensorE matmuls against a ones vector (no 2MB
HBM broadcast); single collective warmup (two would outlast phase 1
and queue-block the real AllGather); epilogue is 8 wide outer-product
groups with stores alternating queues.
"""

import sys

for p in ("/opt/trn_rl_repo", "/root/.axon_site/_ro/trn_rl_repo"):
    if p not in sys.path:
        sys.path.insert(0, p)

import numpy as np

from concourse import bacc, bass, mybir, tile
from concourse.bass_utils import run_bass_kernel_spmd

F32 = mybir.dt.float32
ADD = mybir.AluOpType.add
MULT = mybir.AluOpType.mult
N_CORES = 8
B_FULL, DIN, DOUT = 16384, 4096, 256
B_SHARD = B_FULL // N_CORES          # 2048
M_TILES = B_SHARD // 128             # 16
CLEN = DIN + 3 * DOUT                # cvec: [Bv | A^2 | gamma*A | beta]
BN_EPS = 1e-5
NEG_SLOPE = 0.2

_CACHE = {}
# Bisection/config knobs (HW-debug): each key toggles one v2 feature.
# NOTE: tensor_tensor_reduce hangs TRN2 HW here (sim-only op, bisected
# 2026-08-09) -- phase 1/2 use DVE mult + ACT Copy-accum instead.
_CFG = {
    "stores3d": True, # paired 3D-view output stores (else per-tile 2D)
    "dualq": True,    # x loads alternate sync/scalar queues (else sync only)
    "mmbcast": False, # consts broadcast via TensorE matmul (else DMA broadcast)
    "warmups": 1,     # number of warmup collectives
}
_ACT_FUNC_OVERRIDE = None    # for sim testing (sim lacks Prelu)
_SIM_RSQRT = False  # for sim testing (sim lacks Abs_reciprocal_sqrt)


def _act_func():
    return _ACT_FUNC_OVERRIDE or mybir.ActivationFunctionType.Prelu


def _emit_rsqrt(nc, out_ap, in_ap):
    if _SIM_RSQRT:
        nc.scalar.activation(out_ap, in_ap, mybir.ActivationFunctionType.Sqrt)
        nc.vector.reciprocal(out_ap, out_ap)
    else:
        nc.scalar.activation(
            out_ap, in_ap, mybir.ActivationFunctionType.Abs_reciprocal_sqrt
        )


def _build(variant: str):
    """variant: 'fast' (beta==0, u>=0), 'gnb' (beta==0), 'gbeta'."""
    nc = bacc.Bacc(
        "TRN2",
        target_bir_lowering=False,
        debug=False,
        enable_asserts=False,
        num_devices=N_CORES,
    )
    xs = nc.dram_tensor("xs", [B_SHARD, DIN], F32, kind="ExternalInput").ap()
    cvec = nc.dram_tensor("cvec", [1, CLEN], F32, kind="ExternalInput").ap()
    out = nc.dram_tensor("out", [B_SHARD, DOUT], F32, kind="ExternalOutput").ap()

    with tile.TileContext(nc) as tc:
        with (
            tc.tile_pool(name="xp", bufs=5) as xp,
            tc.tile_pool(name="scr", bufs=3) as scrp,
            tc.tile_pool(name="scr3", bufs=1) as scr3p,
            tc.tile_pool(name="cst", bufs=1) as cst,
            tc.tile_pool(name="op", bufs=3) as op,
            tc.tile_pool(name="ps", bufs=1, space="PSUM") as ps,
            tc.tile_pool(name="dram", bufs=1, space="DRAM") as dram,
        ):
            # Warmup collective(s): absorb CC-stream/mesh first-call
            # setup (~75us) while phase 1 streams x. Result unused.
            for w in range(_CFG["warmups"]):
                wi = dram.tile([2, 1], F32, tag=f"wi{w}")
                wo = dram.tile([2 * N_CORES, 1], F32, tag=f"wo{w}")
                nc.gpsimd.collective_compute(
                    "AllGather",
                    mybir.AluOpType.bypass,
                    replica_groups=[list(range(N_CORES))],
                    ins=[wi.opt()],
                    outs=[wo.opt()],
                )

            # Const strip (19.5KB) on the scalar queue.
            cv_sb = cst.tile([1, CLEN], F32, tag="cv")
            nc.scalar.dma_start(cv_sb[:], cvec[:])

            ones1 = cst.tile([1, 128], F32, tag="ones1")
            nc.vector.memset(ones1[:], 1.0)
            ones128 = cst.tile([128, 1], F32, tag="ones128")
            nc.vector.memset(ones128[:], 1.0)

            # Warm the ACT table set needed at stats time as the FIRST ACT
            # op: the set also contains Copy (filler in every set), so ACT
            # does exactly one TABLE_LOAD for the whole kernel.
            dum = cst.tile([1, 1], F32, tag="dum")
            _emit_rsqrt(nc, dum[:], ones1[0:1, 0:1])
            if variant != "fast":
                dum2 = cst.tile([1, 1], F32, tag="dum2")
                nc.scalar.activation(
                    dum2[:], ones1[0:1, 0:1], _act_func(), alpha=NEG_SLOPE
                )

            # Broadcast consts across partitions via TensorE (rank-1 with
            # a ones row): first the three DOUT vectors, then Bv in two
            # 2048-wide rounds through a 4-bank PSUM tile.
            c_sb = cst.tile([128, 3 * DOUT], F32, tag="c")
            bv_sb = cst.tile([128, DIN], F32, tag="bv")
            if _CFG["mmbcast"]:
                pb = ps.tile([128, 2048], F32, tag="pb")
                nc.tensor.matmul(
                    pb[:, 0:512], ones1[:], cv_sb[0:1, DIN : DIN + 512],
                    start=True, stop=True,
                )
                nc.tensor.matmul(
                    pb[:, 512:768], ones1[:], cv_sb[0:1, DIN + 512 : CLEN],
                    start=True, stop=True,
                )
                nc.vector.tensor_copy(c_sb[:], pb[:, 0:768])
                for half in range(2):
                    pbh = ps.tile([128, 2048], F32, tag="pb")
                    for j in range(4):
                        c0 = half * 2048 + j * 512
                        nc.tensor.matmul(
                            pbh[:, j * 512 : (j + 1) * 512],
                            ones1[:],
                            cv_sb[0:1, c0 : c0 + 512],
                            start=True, stop=True,
                        )
                    nc.vector.tensor_copy(
                        bv_sb[:, half * 2048 : (half + 1) * 2048], pbh[:]
                    )
            else:
                nc.scalar.dma_start(
                    bv_sb[:], cvec[0:1, 0:DIN].broadcast_to([128, DIN])
                )
                nc.scalar.dma_start(
                    c_sb[:],
                    cvec[0:1, DIN:CLEN].broadcast_to([128, 3 * DOUT]),
                )
            a2_b = c_sb[:, 0:DOUT]
            ga_b = c_sb[:, DOUT : 2 * DOUT]
            be_b = c_sb[:, 2 * DOUT : 3 * DOUT]

            t_all = cst.tile([128, M_TILES], F32, tag="t")
            t_parts = cst.tile([128, 4], F32, tag="tparts")

            # Phase 1: DVE mult + ACT Copy-accum per tile; x tiles
            # alternate the two HWDGE queues so per-DMA fixed costs
            # overlap and the SDMA engines never starve.  The last tile
            # is split into 4 column chunks so its mult+reduce pipelines
            # right behind the final DMA instead of adding ~8us serially.
            for m in range(M_TILES):
                x_sb = xp.tile([128, DIN], F32, tag="x")
                eng = nc.sync if (m % 2 == 0 or not _CFG["dualq"]) else nc.scalar
                eng.dma_start(x_sb[:], xs[m * 128 : (m + 1) * 128, :])
                scr = scrp.tile([128, DIN], F32, tag="scr")
                scr3 = scr3p.tile([128, DIN], F32, tag="scr3")
                if m < M_TILES - 1:
                    nc.vector.tensor_mul(scr[:], x_sb[:], bv_sb[:])
                    nc.scalar.activation(
                        scr3[:],
                        scr[:],
                        mybir.ActivationFunctionType.Copy,
                        accum_out=t_all[:, m : m + 1],
                    )
                else:
                    q = DIN // 4
                    for c in range(4):
                        sl = slice(c * q, (c + 1) * q)
                        nc.vector.tensor_mul(scr[:, sl], x_sb[:, sl], bv_sb[:, sl])
                        nc.scalar.activation(
                            scr3[:, sl],
                            scr[:, sl],
                            mybir.ActivationFunctionType.Copy,
                            accum_out=t_parts[:, c : c + 1],
                        )
                    nc.vector.tensor_reduce(
                        out=t_all[:, M_TILES - 1 : M_TILES],
                        in_=t_parts[:],
                        axis=mybir.AxisListType.X,
                        op=ADD,
                    )

            # Phase 2: local partial sums -> cross-partition matmul reduce
            # -> AllGather of [sum_t, sum_t2] per core.
            sp = cst.tile([128, 2], F32, tag="sp")
            nc.vector.tensor_reduce(
                out=sp[:, 0:1], in_=t_all[:], axis=mybir.AxisListType.X, op=ADD
            )
            t_cp = cst.tile([128, M_TILES], F32, tag="tcp")
            nc.vector.tensor_copy(t_cp[:], t_all[:])
            tsq = cst.tile([128, M_TILES], F32, tag="tsq")
            nc.vector.tensor_mul(tsq[:], t_all[:], t_cp[:])
            nc.vector.tensor_reduce(
                out=sp[:, 1:2], in_=tsq[:], axis=mybir.AxisListType.X, op=ADD
            )
            s_ps = ps.tile([2, 1], F32, tag="sps")
            nc.tensor.matmul(s_ps[:], sp[:], ones128[:], start=True, stop=True)
            s_sb = cst.tile([2, 1], F32, tag="ssb")
            nc.vector.tensor_copy(s_sb[:], s_ps[:])

            bi = dram.tile([2, 1], F32, tag="bi")
            bo = dram.tile([2 * N_CORES, 1], F32, tag="bo")
            nc.sync.dma_start(bi[:], s_sb[:])
            nc.gpsimd.collective_compute(
                "AllGather",
                mybir.AluOpType.bypass,
                replica_groups=[list(range(N_CORES))],
                ins=[bi.opt()],
                outs=[bo.opt()],
            )

            # Bring the 16 gathered floats back and replicate across
            # partitions with a tiny matmul (cheaper than a DRE-broadcast
            # DMA), then reduce over ranks with a stride-2 view.
            s16 = cst.tile([1, 2 * N_CORES], F32, tag="s16")
            nc.sync.dma_start(s16[:], bo.rearrange("a b -> b a"))
            pb3 = ps.tile([128, 2 * N_CORES], F32, tag="p3")
            nc.tensor.matmul(pb3[:], ones1[:], s16[:], start=True, stop=True)
            sb16 = cst.tile([128, 2 * N_CORES], F32, tag="sb16")
            nc.vector.tensor_copy(sb16[:], pb3[:])
            sb2 = cst.tile([128, 2], F32, tag="sb2")
            nc.vector.tensor_reduce(
                out=sb2[:],
                in_=sb16.rearrange("p (r s) -> p s r", s=2),
                axis=mybir.AxisListType.X,
                op=ADD,
            )

            # Stats math (replicated on all 128 partitions)
            mcol = cst.tile([128, 1], F32, tag="mcol")
            nc.vector.tensor_scalar_mul(mcol[:], sb2[:, 0:1], 1.0 / B_FULL)
            ecol = cst.tile([128, 1], F32, tag="ecol")
            nc.vector.tensor_scalar_mul(ecol[:], sb2[:, 1:2], 1.0 / B_FULL)
            msq = cst.tile([128, 1], F32, tag="msq")
            nc.vector.tensor_mul(msq[:], mcol[:], mcol[:])
            vcol = cst.tile([128, 1], F32, tag="vcol")
            nc.vector.tensor_sub(vcol[:], ecol[:], msq[:])

            v1 = cst.tile([128, DOUT], F32, tag="v1")
            nc.vector.tensor_scalar(
                v1[:], a2_b, vcol[:, 0:1], BN_EPS, op0=MULT, op1=ADD
            )
            v3 = cst.tile([128, DOUT], F32, tag="v3")
            _emit_rsqrt(nc, v3[:], v1[:])
            u_b = cst.tile([128, DOUT], F32, tag="ub")
            nc.vector.tensor_mul(u_b[:], v3[:], ga_b)

            tcall = cst.tile([128, M_TILES], F32, tag="tc")
            nc.vector.tensor_scalar_sub(tcall[:], t_all[:], mcol[:, 0:1])

            if variant == "fast":
                # lrelu commutes with the positive per-column scale u:
                # apply it to the tiny t column once, then phase 3 is a
                # pure outer product on DVE.
                tneg = cst.tile([128, M_TILES], F32, tag="tn")
                nc.vector.tensor_scalar_mul(tneg[:], tcall[:], NEG_SLOPE)
                tl = cst.tile([128, M_TILES], F32, tag="tl")
                nc.vector.tensor_tensor(
                    tl[:], tcall[:], tneg[:], op=mybir.AluOpType.max
                )
            else:
                tl = tcall

            # Phase 3: 8 groups of 2 row-tiles, stores alternate queues.
            for g in range(M_TILES // 2):
                ow = op.tile([128, 2 * DOUT], F32, tag="ow")
                for b in range(2):
                    m = 2 * g + b
                    dst = ow[:, b * DOUT : (b + 1) * DOUT]
                    if variant == "fast":
                        nc.vector.tensor_scalar_mul(dst, u_b[:], tl[:, m : m + 1])
                    elif variant == "gnb":
                        nc.scalar.activation(
                            dst, u_b[:], _act_func(),
                            scale=tl[:, m : m + 1], alpha=NEG_SLOPE,
                        )
                    else:  # gbeta
                        y = op.tile([128, DOUT], F32, tag="y")
                        nc.vector.scalar_tensor_tensor(
                            out=y[:],
                            in0=u_b[:],
                            scalar=tl[:, m : m + 1],
                            in1=be_b,
                            op0=MULT,
                            op1=ADD,
                        )
                        nc.scalar.activation(dst, y[:], _act_func(), alpha=NEG_SLOPE)
                dma_eng = nc.sync if g % 2 == 0 else nc.scalar
                if _CFG["stores3d"]:
                    dma_eng.dma_start(
                        out[g * 256 : (g + 1) * 256, :].rearrange(
                            "(b p) f -> p b f", p=128
                        ),
                        ow.rearrange("p (b f) -> p b f", b=2),
                    )
                else:
                    for b in range(2):
                        m = 2 * g + b
                        dma_eng.dma_start(
                            out[m * 128 : (m + 1) * 128, :],
                            ow[:, b * DOUT : (b + 1) * DOUT],
                        )

    nc.compile()
    return nc


def _get_nc(variant: str):
    key = (variant, tuple(sorted(_CFG.items())))
    if key not in _CACHE:
        _CACHE[key] = _build(variant)
    return _CACHE[key]


def kernel(x, alphas_A, controls_A, controls_B, linear_bias, bn_gamma, bn_beta,
           _trace=False):
    x = np.asarray(x, dtype=np.float32)
    alphas_A = np.asarray(alphas_A, dtype=np.float32)
    controls_A = np.asarray(controls_A, dtype=np.float32)
    controls_B = np.asarray(controls_B, dtype=np.float32)
    bn_gamma = np.asarray(bn_gamma, dtype=np.float32)
    bn_beta = np.asarray(bn_beta, dtype=np.float32)

    A = (alphas_A * controls_A).sum(axis=0).astype(np.float32)          # (256,)
    Bv = (controls_B * alphas_A.T).sum(axis=1).astype(np.float32)       # (4096,)
    ga = (bn_gamma * A).astype(np.float32)
    cvec = np.concatenate([Bv, A * A, ga, bn_beta]).reshape(1, CLEN)
    cvec = np.ascontiguousarray(cvec, dtype=np.float32)

    with_beta = bool(np.any(bn_beta != 0))
    if with_beta:
        variant = "gbeta"
    elif bool(np.all(ga >= 0)):
        variant = "fast"
    else:
        variant = "gnb"

    nc = _get_nc(variant)
    in_maps = []
    for c in range(N_CORES):
        in_maps.append({
            "xs": np.ascontiguousarray(x[c * B_SHARD : (c + 1) * B_SHARD]),
            "cvec": cvec,
        })

    res = run_bass_kernel_spmd(
        nc, in_maps, core_ids=list(range(N_CORES)), trace=_trace
    )
    out = np.concatenate([r["out"] for r in res.results], axis=0)
    if _trace:
        return out, res
    return out


# revision 12
# speedup vs baseline: 1.1210x; 1.1210x over previous
"""Trainium2 Bass kernel for nn_MLPLoraSubspace.

Math: A = sum(alphas_A * controls_A, 0)  (256,)
      Bv = sum(alphas_A.T * controls_B, 1)  (4096,)
      W = A outer Bv  (rank-1)  -> out = (x @ Bv) outer A + bias
      BatchNorm(training stats) then LeakyReLU(0.2).

Because W is rank-1, out[i,j] = A[j]*t[i] + bias[j] with t = x @ Bv.
Batch stats:  mean_j = A_j*mean(t) + bias_j,  var_j = A_j^2*var(t), so
  act[i,j] = lrelu( u_j * (t[i]-mean_t) + beta_j ),
  u_j = gamma_j*A_j/sqrt(A_j^2*var_t+eps).  Bias cancels exactly.

Fast path (beta==0 and gamma*A >= 0, true for the reference inputs):
  lrelu(u_j * z) = u_j * lrelu(z), so the activation collapses onto the
  tiny t column and phase 3 is a pure outer product.

Sharding: data-parallel over batch, 8 cores x 2048 rows. Per-core partial
(sum t, sum t^2) is AllGather'd (2 floats) to form global batch stats.

v2 layout: x streamed as 16x 2MB tile DMAs alternating the two HWDGE
queues (sync/scalar); per tile ONE DVE tensor_tensor_reduce does
mult+rowsum; consts arrive as one [1,4864] strip and are broadcast
across partitions with TensorE matmuls against a ones vector (no 2MB
HBM broadcast); single collective warmup (two would outlast phase 1
and queue-block the real AllGather); epilogue is 8 wide outer-product
groups with stores alternating queues.
"""

import sys

for p in ("/opt/trn_rl_repo", "/root/.axon_site/_ro/trn_rl_repo"):
    if p not in sys.path:
        sys.path.insert(0, p)

import numpy as np

from concourse import bacc, bass, mybir, tile
from concourse.bass_utils import run_bass_kernel_spmd

F32 = mybir.dt.float32
BF16 = mybir.dt.bfloat16
ADD = mybir.AluOpType.add
MULT = mybir.AluOpType.mult
N_CORES = 8
B_FULL, DIN, DOUT = 16384, 4096, 256
B_SHARD = B_FULL // N_CORES          # 2048
M_TILES = B_SHARD // 128             # 16
CLEN = DIN + 3 * DOUT                # cvec: [Bv | A^2 | gamma*A | beta]
BN_EPS = 1e-5
NEG_SLOPE = 0.2

_CACHE = {}
# Bisection/config knobs (HW-debug): each key toggles one v2 feature.
# NOTE: tensor_tensor_reduce hangs TRN2 HW here (sim-only op, bisected
# 2026-08-09) -- phase 1/2 use DVE mult + ACT Copy-accum instead.
_CFG = {
    "stores3d": True, # paired 3D-view output stores (else per-tile 2D)
    "dualq": True,    # x loads alternate sync/scalar queues (else sync only)
    "warmups": 1,     # number of warmup collectives
}
_ACT_FUNC_OVERRIDE = None    # for sim testing (sim lacks Prelu)
_SIM_RSQRT = False  # for sim testing (sim lacks Abs_reciprocal_sqrt)


def _act_func():
    return _ACT_FUNC_OVERRIDE or mybir.ActivationFunctionType.Prelu


def _emit_rsqrt(nc, out_ap, in_ap):
    if _SIM_RSQRT:
        nc.scalar.activation(out_ap, in_ap, mybir.ActivationFunctionType.Sqrt)
        nc.vector.reciprocal(out_ap, out_ap)
    else:
        nc.scalar.activation(
            out_ap, in_ap, mybir.ActivationFunctionType.Abs_reciprocal_sqrt
        )


def _build(variant: str):
    """variant: 'fast' (beta==0, u>=0), 'gnb' (beta==0), 'gbeta'."""
    nc = bacc.Bacc(
        "TRN2",
        target_bir_lowering=False,
        debug=False,
        enable_asserts=False,
        num_devices=N_CORES,
    )
    xs = nc.dram_tensor("xs", [B_SHARD, DIN], BF16, kind="ExternalInput").ap()
    cbv = nc.dram_tensor("cbv", [1, DIN], BF16, kind="ExternalInput").ap()
    cf = nc.dram_tensor("cf", [1, 3 * DOUT], F32, kind="ExternalInput").ap()
    out = nc.dram_tensor("out", [B_SHARD, DOUT], F32, kind="ExternalOutput").ap()

    with tile.TileContext(nc) as tc:
        with (
            tc.tile_pool(name="xp", bufs=8) as xp,
            tc.tile_pool(name="scr", bufs=3) as scrp,
            tc.tile_pool(name="scr3", bufs=1) as scr3p,
            tc.tile_pool(name="cst", bufs=1) as cst,
            tc.tile_pool(name="op", bufs=3) as op,
            tc.tile_pool(name="ps", bufs=1, space="PSUM") as ps,
            tc.tile_pool(name="dram", bufs=1, space="DRAM") as dram,
        ):
            # Warmup collective(s): absorb CC-stream/mesh first-call
            # setup (~75us) while phase 1 streams x. Result unused.
            for w in range(_CFG["warmups"]):
                wi = dram.tile([2, 1], F32, tag=f"wi{w}")
                wo = dram.tile([2 * N_CORES, 1], F32, tag=f"wo{w}")
                nc.gpsimd.collective_compute(
                    "AllGather",
                    mybir.AluOpType.bypass,
                    replica_groups=[list(range(N_CORES))],
                    ins=[wi.opt()],
                    outs=[wo.opt()],
                )

            # Consts: Bv (bf16) broadcast in two halves front-loaded on
            # BOTH HWDGE queues; the f32 [a2|ga|beta] strip behind the
            # scalar half.  All land by ~10us so phase-1 compute starts
            # immediately; x tiles queue right behind.
            bv_sb = cst.tile([128, DIN], BF16, tag="bv")
            nc.sync.dma_start(
                bv_sb[:, 0 : DIN // 2],
                cbv[0:1, 0 : DIN // 2].broadcast_to([128, DIN // 2]),
            )
            nc.scalar.dma_start(
                bv_sb[:, DIN // 2 : DIN],
                cbv[0:1, DIN // 2 : DIN].broadcast_to([128, DIN // 2]),
            )
            c_sb = cst.tile([128, 3 * DOUT], F32, tag="c")
            nc.scalar.dma_start(
                c_sb[:], cf[0:1, :].broadcast_to([128, 3 * DOUT])
            )

            ones1 = cst.tile([1, 128], F32, tag="ones1")
            nc.vector.memset(ones1[:], 1.0)
            ones128 = cst.tile([128, 1], F32, tag="ones128")
            nc.vector.memset(ones128[:], 1.0)

            # Warm the ACT table set needed at stats time as the FIRST ACT
            # op: the set also contains Copy (filler in every set), so ACT
            # does exactly one TABLE_LOAD for the whole kernel.
            dum = cst.tile([1, 1], F32, tag="dum")
            _emit_rsqrt(nc, dum[:], ones1[0:1, 0:1])
            if variant != "fast":
                dum2 = cst.tile([1, 1], F32, tag="dum2")
                nc.scalar.activation(
                    dum2[:], ones1[0:1, 0:1], _act_func(), alpha=NEG_SLOPE
                )

            a2_b = c_sb[:, 0:DOUT]
            ga_b = c_sb[:, DOUT : 2 * DOUT]
            be_b = c_sb[:, 2 * DOUT : 3 * DOUT]

            t_all = cst.tile([128, M_TILES], F32, tag="t")
            t_parts = cst.tile([128, 4], F32, tag="tparts")

            # Phase 1: DVE mult + ACT Copy-accum per tile; x tiles
            # alternate the two HWDGE queues so per-DMA fixed costs
            # overlap and the SDMA engines never starve.  The last tile
            # is split into 4 column chunks so its mult+reduce pipelines
            # right behind the final DMA instead of adding ~8us serially.
            for m in range(M_TILES):
                x_sb = xp.tile([128, DIN], BF16, tag="x")
                eng = nc.sync if (m % 2 == 0 or not _CFG["dualq"]) else nc.scalar
                eng.dma_start(x_sb[:], xs[m * 128 : (m + 1) * 128, :])
                scr = scrp.tile([128, DIN], BF16, tag="scr")
                scr3 = scr3p.tile([128, DIN], BF16, tag="scr3")
                if m < M_TILES - 1:
                    nc.vector.tensor_mul(scr[:], x_sb[:], bv_sb[:])
                    nc.scalar.activation(
                        scr3[:],
                        scr[:],
                        mybir.ActivationFunctionType.Copy,
                        accum_out=t_all[:, m : m + 1],
                    )
                else:
                    q = DIN // 4
                    for c in range(4):
                        sl = slice(c * q, (c + 1) * q)
                        nc.vector.tensor_mul(scr[:, sl], x_sb[:, sl], bv_sb[:, sl])
                        nc.scalar.activation(
                            scr3[:, sl],
                            scr[:, sl],
                            mybir.ActivationFunctionType.Copy,
                            accum_out=t_parts[:, c : c + 1],
                        )
                    nc.vector.tensor_reduce(
                        out=t_all[:, M_TILES - 1 : M_TILES],
                        in_=t_parts[:],
                        axis=mybir.AxisListType.X,
                        op=ADD,
                    )

            # Phase 2: local partial sums -> cross-partition matmul reduce
            # -> AllGather of [sum_t, sum_t2] per core.
            sp = cst.tile([128, 2], F32, tag="sp")
            nc.vector.tensor_reduce(
                out=sp[:, 0:1], in_=t_all[:], axis=mybir.AxisListType.X, op=ADD
            )
            t_cp = cst.tile([128, M_TILES], F32, tag="tcp")
            nc.vector.tensor_copy(t_cp[:], t_all[:])
            tsq = cst.tile([128, M_TILES], F32, tag="tsq")
            nc.vector.tensor_mul(tsq[:], t_all[:], t_cp[:])
            nc.vector.tensor_reduce(
                out=sp[:, 1:2], in_=tsq[:], axis=mybir.AxisListType.X, op=ADD
            )
            s_ps = ps.tile([2, 1], F32, tag="sps")
            nc.tensor.matmul(s_ps[:], sp[:], ones128[:], start=True, stop=True)
            s_sb = cst.tile([2, 1], F32, tag="ssb")
            nc.vector.tensor_copy(s_sb[:], s_ps[:])

            bi = dram.tile([2, 1], F32, tag="bi")
            bo = dram.tile([2 * N_CORES, 1], F32, tag="bo")
            nc.sync.dma_start(bi[:], s_sb[:])
            nc.gpsimd.collective_compute(
                "AllGather",
                mybir.AluOpType.bypass,
                replica_groups=[list(range(N_CORES))],
                ins=[bi.opt()],
                outs=[bo.opt()],
            )

            # Bring the 16 gathered floats back and replicate across
            # partitions with a tiny matmul (cheaper than a DRE-broadcast
            # DMA), then reduce over ranks with a stride-2 view.
            s16 = cst.tile([1, 2 * N_CORES], F32, tag="s16")
            nc.sync.dma_start(s16[:], bo.rearrange("a b -> b a"))
            pb3 = ps.tile([128, 2 * N_CORES], F32, tag="p3")
            nc.tensor.matmul(pb3[:], ones1[:], s16[:], start=True, stop=True)
            sb16 = cst.tile([128, 2 * N_CORES], F32, tag="sb16")
            nc.vector.tensor_copy(sb16[:], pb3[:])
            sb2 = cst.tile([128, 2], F32, tag="sb2")
            nc.vector.tensor_reduce(
                out=sb2[:],
                in_=sb16.rearrange("p (r s) -> p s r", s=2),
                axis=mybir.AxisListType.X,
                op=ADD,
            )

            # Stats math (replicated on all 128 partitions)
            mcol = cst.tile([128, 1], F32, tag="mcol")
            nc.vector.tensor_scalar_mul(mcol[:], sb2[:, 0:1], 1.0 / B_FULL)
            ecol = cst.tile([128, 1], F32, tag="ecol")
            nc.vector.tensor_scalar_mul(ecol[:], sb2[:, 1:2], 1.0 / B_FULL)
            msq = cst.tile([128, 1], F32, tag="msq")
            nc.vector.tensor_mul(msq[:], mcol[:], mcol[:])
            vcol = cst.tile([128, 1], F32, tag="vcol")
            nc.vector.tensor_sub(vcol[:], ecol[:], msq[:])

            v1 = cst.tile([128, DOUT], F32, tag="v1")
            nc.vector.tensor_scalar(
                v1[:], a2_b, vcol[:, 0:1], BN_EPS, op0=MULT, op1=ADD
            )
            v3 = cst.tile([128, DOUT], F32, tag="v3")
            _emit_rsqrt(nc, v3[:], v1[:])
            u_b = cst.tile([128, DOUT], F32, tag="ub")
            nc.vector.tensor_mul(u_b[:], v3[:], ga_b)

            tcall = cst.tile([128, M_TILES], F32, tag="tc")
            nc.vector.tensor_scalar_sub(tcall[:], t_all[:], mcol[:, 0:1])

            if variant == "fast":
                # lrelu commutes with the positive per-column scale u:
                # apply it to the tiny t column once, then phase 3 is a
                # pure outer product on DVE.
                tneg = cst.tile([128, M_TILES], F32, tag="tn")
                nc.vector.tensor_scalar_mul(tneg[:], tcall[:], NEG_SLOPE)
                tl = cst.tile([128, M_TILES], F32, tag="tl")
                nc.vector.tensor_tensor(
                    tl[:], tcall[:], tneg[:], op=mybir.AluOpType.max
                )
            else:
                tl = tcall

            # Phase 3: 8 groups of 2 row-tiles, stores alternate queues.
            for g in range(M_TILES // 2):
                ow = op.tile([128, 2 * DOUT], F32, tag="ow")
                for b in range(2):
                    m = 2 * g + b
                    dst = ow[:, b * DOUT : (b + 1) * DOUT]
                    if variant == "fast":
                        if m % 3 == 2:
                            # offload ~1/3 of the outer products to ACT
                            nc.scalar.activation(
                                dst, u_b[:],
                                mybir.ActivationFunctionType.Copy,
                                scale=tl[:, m : m + 1],
                            )
                        else:
                            nc.vector.tensor_scalar_mul(
                                dst, u_b[:], tl[:, m : m + 1]
                            )
                    elif variant == "gnb":
                        nc.scalar.activation(
                            dst, u_b[:], _act_func(),
                            scale=tl[:, m : m + 1], alpha=NEG_SLOPE,
                        )
                    else:  # gbeta
                        y = op.tile([128, DOUT], F32, tag="y")
                        nc.vector.scalar_tensor_tensor(
                            out=y[:],
                            in0=u_b[:],
                            scalar=tl[:, m : m + 1],
                            in1=be_b,
                            op0=MULT,
                            op1=ADD,
                        )
                        nc.scalar.activation(dst, y[:], _act_func(), alpha=NEG_SLOPE)
                dma_eng = nc.sync if g % 2 == 0 else nc.scalar
                if _CFG["stores3d"]:
                    dma_eng.dma_start(
                        out[g * 256 : (g + 1) * 256, :].rearrange(
                            "(b p) f -> p b f", p=128
                        ),
                        ow.rearrange("p (b f) -> p b f", b=2),
                    )
                else:
                    for b in range(2):
                        m = 2 * g + b
                        dma_eng.dma_start(
                            out[m * 128 : (m + 1) * 128, :],
                            ow[:, b * DOUT : (b + 1) * DOUT],
                        )

    nc.compile()
    return nc


def _get_nc(variant: str):
    key = (variant, tuple(sorted(_CFG.items())))
    if key not in _CACHE:
        _CACHE[key] = _build(variant)
    return _CACHE[key]


def kernel(x, alphas_A, controls_A, controls_B, linear_bias, bn_gamma, bn_beta,
           _trace=False):
    x = np.asarray(x, dtype=np.float32)
    alphas_A = np.asarray(alphas_A, dtype=np.float32)
    controls_A = np.asarray(controls_A, dtype=np.float32)
    controls_B = np.asarray(controls_B, dtype=np.float32)
    bn_gamma = np.asarray(bn_gamma, dtype=np.float32)
    bn_beta = np.asarray(bn_beta, dtype=np.float32)

    import ml_dtypes

    A = (alphas_A * controls_A).sum(axis=0).astype(np.float32)          # (256,)
    Bv = (controls_B * alphas_A.T).sum(axis=1).astype(np.float32)       # (4096,)
    ga = (bn_gamma * A).astype(np.float32)
    cbv = np.ascontiguousarray(
        Bv.reshape(1, DIN).astype(ml_dtypes.bfloat16))
    cf = np.ascontiguousarray(
        np.concatenate([A * A, ga, bn_beta]).reshape(1, 3 * DOUT),
        dtype=np.float32)
    x_bf = np.ascontiguousarray(x.astype(ml_dtypes.bfloat16))

    with_beta = bool(np.any(bn_beta != 0))
    if with_beta:
        variant = "gbeta"
    elif bool(np.all(ga >= 0)):
        variant = "fast"
    else:
        variant = "gnb"

    nc = _get_nc(variant)
    in_maps = []
    for c in range(N_CORES):
        in_maps.append({
            "xs": np.ascontiguousarray(x_bf[c * B_SHARD : (c + 1) * B_SHARD]),
            "cbv": cbv,
            "cf": cf,
        })

    res = run_bass_kernel_spmd(
        nc, in_maps, core_ids=list(range(N_CORES)), trace=_trace
    )
    out = np.concatenate([r["out"] for r in res.results], axis=0)
    if _trace:
        return out, res
    return out


# revision 14
# speedup vs baseline: 1.3611x; 1.2142x over previous
"""Trainium2 Bass kernel for nn_MLPLoraSubspace.

Math: A = sum(alphas_A * controls_A, 0)  (256,)
      Bv = sum(alphas_A.T * controls_B, 1)  (4096,)
      W = A outer Bv  (rank-1)  -> out = (x @ Bv) outer A + bias
      BatchNorm(training stats) then LeakyReLU(0.2).

Because W is rank-1, out[i,j] = A[j]*t[i] + bias[j] with t = x @ Bv.
Batch stats:  mean_j = A_j*mean(t) + bias_j,  var_j = A_j^2*var(t), so
  act[i,j] = lrelu( u_j * (t[i]-mean_t) + beta_j ),
  u_j = gamma_j*A_j/sqrt(A_j^2*var_t+eps).  Bias cancels exactly.

Fast path (beta==0 and gamma*A >= 0, true for the reference inputs):
  lrelu(u_j * z) = u_j * lrelu(z), so the activation collapses onto the
  tiny t column and phase 3 is a pure outer product.

Sharding: data-parallel over batch, 8 cores x 2048 rows. Per-core partial
(sum t, sum t^2) is AllGather'd (2 floats) to form global batch stats.

v2 layout: x streamed as 16x 2MB tile DMAs alternating the two HWDGE
queues (sync/scalar); per tile ONE DVE tensor_tensor_reduce does
mult+rowsum; consts arrive as one [1,4864] strip and are broadcast
across partitions with TensorE matmuls against a ones vector (no 2MB
HBM broadcast); single collective warmup (two would outlast phase 1
and queue-block the real AllGather); epilogue is 8 wide outer-product
groups with stores alternating queues.
"""

import sys

for p in ("/opt/trn_rl_repo", "/root/.axon_site/_ro/trn_rl_repo"):
    if p not in sys.path:
        sys.path.insert(0, p)

import numpy as np

from concourse import bacc, bass, mybir, tile
from concourse.bass_utils import run_bass_kernel_spmd

F32 = mybir.dt.float32
BF16 = mybir.dt.bfloat16
ADD = mybir.AluOpType.add
MULT = mybir.AluOpType.mult
N_CORES = 8
B_FULL, DIN, DOUT = 16384, 4096, 256
B_SHARD = B_FULL // N_CORES          # 2048
M_TILES = B_SHARD // 128             # 16
CLEN = DIN + 3 * DOUT                # cvec: [Bv | A^2 | gamma*A | beta]
BN_EPS = 1e-5
NEG_SLOPE = 0.2

_CACHE = {}
# Bisection/config knobs (HW-debug): each key toggles one v2 feature.
# NOTE: tensor_tensor_reduce hangs TRN2 HW here (sim-only op, bisected
# 2026-08-09) -- phase 1/2 use DVE mult + ACT Copy-accum instead.
_CFG = {
    "stores3d": True, # paired 3D-view output stores (else per-tile 2D)
    "dualq": True,    # x loads alternate sync/scalar queues (else sync only)
    "warmups": 1,     # number of warmup collectives
    "nocc": False,    # timing probe: skip collectives, use local sums (WRONG output)
}
_ACT_FUNC_OVERRIDE = None    # for sim testing (sim lacks Prelu)
_SIM_RSQRT = False  # for sim testing (sim lacks Abs_reciprocal_sqrt)


def _act_func():
    return _ACT_FUNC_OVERRIDE or mybir.ActivationFunctionType.Prelu


def _emit_rsqrt(nc, out_ap, in_ap):
    if _SIM_RSQRT:
        nc.scalar.activation(out_ap, in_ap, mybir.ActivationFunctionType.Sqrt)
        nc.vector.reciprocal(out_ap, out_ap)
    else:
        nc.scalar.activation(
            out_ap, in_ap, mybir.ActivationFunctionType.Abs_reciprocal_sqrt
        )


def _build(variant: str):
    """variant: 'fast' (beta==0, u>=0), 'gnb' (beta==0), 'gbeta'."""
    nc = bacc.Bacc(
        "TRN2",
        target_bir_lowering=False,
        debug=False,
        enable_asserts=False,
        num_devices=N_CORES,
    )
    xs = nc.dram_tensor("xs", [B_SHARD, DIN], BF16, kind="ExternalInput").ap()
    cbv = nc.dram_tensor("cbv", [1, DIN], BF16, kind="ExternalInput").ap()
    cf = nc.dram_tensor("cf", [1, 3 * DOUT], F32, kind="ExternalInput").ap()
    out = nc.dram_tensor("out", [B_SHARD, DOUT], F32, kind="ExternalOutput").ap()

    with tile.TileContext(nc) as tc:
        with (
            tc.tile_pool(name="xp", bufs=8) as xp,
            tc.tile_pool(name="scr", bufs=3) as scrp,
            tc.tile_pool(name="scr3", bufs=1) as scr3p,
            tc.tile_pool(name="cst", bufs=1) as cst,
            tc.tile_pool(name="op", bufs=8) as op,
            tc.tile_pool(name="ps", bufs=1, space="PSUM") as ps,
            tc.tile_pool(name="dram", bufs=1, space="DRAM") as dram,
        ):
            # Warmup collective(s): absorb CC-stream/mesh first-call
            # setup (~75us) while phase 1 streams x. Result unused.
            for w in range(0 if _CFG["nocc"] else _CFG["warmups"]):
                wi = dram.tile([2, 1], F32, tag=f"wi{w}")
                wo = dram.tile([2 * N_CORES, 1], F32, tag=f"wo{w}")
                nc.gpsimd.collective_compute(
                    "AllGather",
                    mybir.AluOpType.bypass,
                    replica_groups=[list(range(N_CORES))],
                    ins=[wi.opt()],
                    outs=[wo.opt()],
                )

            # Consts: Bv (bf16) broadcast in two halves front-loaded on
            # BOTH HWDGE queues; the f32 [a2|ga|beta] strip behind the
            # scalar half.  All land by ~10us so phase-1 compute starts
            # immediately; x tiles queue right behind.
            bv_sb = cst.tile([128, DIN], BF16, tag="bv")
            nc.sync.dma_start(
                bv_sb[:, 0 : DIN // 2],
                cbv[0:1, 0 : DIN // 2].broadcast_to([128, DIN // 2]),
            )
            nc.scalar.dma_start(
                bv_sb[:, DIN // 2 : DIN],
                cbv[0:1, DIN // 2 : DIN].broadcast_to([128, DIN // 2]),
            )
            c_sb = cst.tile([128, 3 * DOUT], F32, tag="c")
            nc.scalar.dma_start(
                c_sb[:], cf[0:1, :].broadcast_to([128, 3 * DOUT])
            )

            ones1 = cst.tile([1, 128], F32, tag="ones1")
            nc.vector.memset(ones1[:], 1.0)
            ones128 = cst.tile([128, 1], F32, tag="ones128")
            nc.vector.memset(ones128[:], 1.0)

            # Warm the ACT table set needed at stats time as the FIRST ACT
            # op: the set also contains Copy (filler in every set), so ACT
            # does exactly one TABLE_LOAD for the whole kernel.
            dum = cst.tile([1, 1], F32, tag="dum")
            _emit_rsqrt(nc, dum[:], ones1[0:1, 0:1])
            if variant != "fast":
                dum2 = cst.tile([1, 1], F32, tag="dum2")
                nc.scalar.activation(
                    dum2[:], ones1[0:1, 0:1], _act_func(), alpha=NEG_SLOPE
                )

            a2_b = c_sb[:, 0:DOUT]
            ga_b = c_sb[:, DOUT : 2 * DOUT]
            be_b = c_sb[:, 2 * DOUT : 3 * DOUT]

            t_all = cst.tile([128, M_TILES], F32, tag="t")
            t_a = cst.tile([128, M_TILES], F32, tag="ta")
            t_b = cst.tile([128, M_TILES], F32, tag="tb")
            t_parts = cst.tile([128, 4], F32, tag="tparts")
            ASPLIT = 13 * 256  # ACT reduces cols [0:ASPLIT), DVE the rest

            # Phase 1: DVE mult + ACT Copy-accum per tile; x tiles
            # alternate the two HWDGE queues so per-DMA fixed costs
            # overlap and the SDMA engines never starve.  The last tile
            # is split into 4 column chunks so its mult+reduce pipelines
            # right behind the final DMA instead of adding ~8us serially.
            for m in range(M_TILES):
                x_sb = xp.tile([128, DIN], BF16, tag="x")
                eng = nc.sync if (m % 2 == 0 or not _CFG["dualq"]) else nc.scalar
                eng.dma_start(x_sb[:], xs[m * 128 : (m + 1) * 128, :])
                scr = scrp.tile([128, DIN], BF16, tag="scr")
                scr3 = scr3p.tile([128, DIN], BF16, tag="scr3")
                if m < M_TILES - 1:
                    nc.vector.tensor_mul(scr[:], x_sb[:], bv_sb[:])
                    nc.scalar.activation(
                        scr3[:, 0:ASPLIT],
                        scr[:, 0:ASPLIT],
                        mybir.ActivationFunctionType.Copy,
                        accum_out=t_a[:, m : m + 1],
                    )
                    nc.vector.tensor_reduce(
                        out=t_b[:, m : m + 1],
                        in_=scr[:, ASPLIT:DIN],
                        axis=mybir.AxisListType.X,
                        op=ADD,
                    )
                else:
                    q = DIN // 4
                    for c in range(4):
                        sl = slice(c * q, (c + 1) * q)
                        nc.vector.tensor_mul(scr[:, sl], x_sb[:, sl], bv_sb[:, sl])
                        if c < 3:
                            nc.scalar.activation(
                                scr3[:, sl],
                                scr[:, sl],
                                mybir.ActivationFunctionType.Copy,
                                accum_out=t_parts[:, c : c + 1],
                            )
                        else:
                            nc.vector.tensor_reduce(
                                out=t_parts[:, c : c + 1],
                                in_=scr[:, sl],
                                axis=mybir.AxisListType.X,
                                op=ADD,
                            )
                    nc.vector.tensor_add(
                        t_a[:, M_TILES - 1 : M_TILES],
                        t_parts[:, 0:1],
                        t_parts[:, 1:2],
                    )
                    nc.vector.tensor_add(
                        t_b[:, M_TILES - 1 : M_TILES],
                        t_parts[:, 2:3],
                        t_parts[:, 3:4],
                    )
            nc.vector.tensor_add(t_all[:], t_a[:], t_b[:])

            # Phase 2: local partial sums -> cross-partition matmul reduce
            # -> AllGather of [sum_t, sum_t2] per core.
            sp = cst.tile([128, 2], F32, tag="sp")
            nc.vector.tensor_reduce(
                out=sp[:, 0:1], in_=t_all[:], axis=mybir.AxisListType.X, op=ADD
            )
            t_cp = cst.tile([128, M_TILES], F32, tag="tcp")
            nc.vector.tensor_copy(t_cp[:], t_all[:])
            tsq = cst.tile([128, M_TILES], F32, tag="tsq")
            nc.vector.tensor_mul(tsq[:], t_all[:], t_cp[:])
            nc.vector.tensor_reduce(
                out=sp[:, 1:2], in_=tsq[:], axis=mybir.AxisListType.X, op=ADD
            )
            s_ps = ps.tile([2, 1], F32, tag="sps")
            nc.tensor.matmul(s_ps[:], sp[:], ones128[:], start=True, stop=True)
            s_sb = cst.tile([2, 1], F32, tag="ssb")
            nc.vector.tensor_copy(s_sb[:], s_ps[:])

            sb2 = cst.tile([128, 2], F32, tag="sb2")
            if _CFG["nocc"]:
                # timing probe only: pretend local sums are global
                nc.vector.tensor_scalar_mul(sb2[:], sp[:], float(N_CORES))
            else:
                bi = dram.tile([2, 1], F32, tag="bi")
                bo = dram.tile([2 * N_CORES, 1], F32, tag="bo")
                nc.sync.dma_start(bi[:], s_sb[:])
                nc.gpsimd.collective_compute(
                    "AllGather",
                    mybir.AluOpType.bypass,
                    replica_groups=[list(range(N_CORES))],
                    ins=[bi.opt()],
                    outs=[bo.opt()],
                )

                # Bring the 16 gathered floats back and replicate across
                # partitions with a tiny matmul (cheaper than a
                # DRE-broadcast DMA), then reduce over ranks with a
                # stride-2 view.
                s16 = cst.tile([1, 2 * N_CORES], F32, tag="s16")
                nc.sync.dma_start(s16[:], bo.rearrange("a b -> b a"))
                pb3 = ps.tile([128, 2 * N_CORES], F32, tag="p3")
                nc.tensor.matmul(
                    pb3[:], ones1[:], s16[:], start=True, stop=True
                )
                sb16 = cst.tile([128, 2 * N_CORES], F32, tag="sb16")
                nc.vector.tensor_copy(sb16[:], pb3[:])
                nc.vector.tensor_reduce(
                    out=sb2[:],
                    in_=sb16.rearrange("p (r s) -> p s r", s=2),
                    axis=mybir.AxisListType.X,
                    op=ADD,
                )

            # Stats math (replicated on all 128 partitions)
            mcol = cst.tile([128, 1], F32, tag="mcol")
            nc.vector.tensor_scalar_mul(mcol[:], sb2[:, 0:1], 1.0 / B_FULL)
            ecol = cst.tile([128, 1], F32, tag="ecol")
            nc.vector.tensor_scalar_mul(ecol[:], sb2[:, 1:2], 1.0 / B_FULL)
            msq = cst.tile([128, 1], F32, tag="msq")
            nc.vector.tensor_mul(msq[:], mcol[:], mcol[:])
            vcol = cst.tile([128, 1], F32, tag="vcol")
            nc.vector.tensor_sub(vcol[:], ecol[:], msq[:])

            v1 = cst.tile([128, DOUT], F32, tag="v1")
            nc.vector.tensor_scalar(
                v1[:], a2_b, vcol[:, 0:1], BN_EPS, op0=MULT, op1=ADD
            )
            v3 = cst.tile([128, DOUT], F32, tag="v3")
            _emit_rsqrt(nc, v3[:], v1[:])
            u_b = cst.tile([128, DOUT], F32, tag="ub")
            nc.vector.tensor_mul(u_b[:], v3[:], ga_b)

            tcall = cst.tile([128, M_TILES], F32, tag="tc")
            nc.vector.tensor_scalar_sub(tcall[:], t_all[:], mcol[:, 0:1])

            if variant == "fast":
                # lrelu commutes with the positive per-column scale u:
                # apply it to the tiny t column once, then phase 3 is a
                # pure outer product on DVE.
                tneg = cst.tile([128, M_TILES], F32, tag="tn")
                nc.vector.tensor_scalar_mul(tneg[:], tcall[:], NEG_SLOPE)
                tl = cst.tile([128, M_TILES], F32, tag="tl")
                nc.vector.tensor_tensor(
                    tl[:], tcall[:], tneg[:], op=mybir.AluOpType.max
                )
            else:
                tl = tcall

            # Phase 3: 8 groups of 2 row-tiles, stores alternate queues.
            for g in range(M_TILES // 2):
                ow = op.tile([128, 2 * DOUT], F32, tag="ow")
                for b in range(2):
                    m = 2 * g + b
                    dst = ow[:, b * DOUT : (b + 1) * DOUT]
                    if variant == "fast":
                        if m % 3 == 2:
                            # offload ~1/3 of the outer products to ACT
                            nc.scalar.activation(
                                dst, u_b[:],
                                mybir.ActivationFunctionType.Copy,
                                scale=tl[:, m : m + 1],
                            )
                        else:
                            nc.vector.tensor_scalar_mul(
                                dst, u_b[:], tl[:, m : m + 1]
                            )
                    elif variant == "gnb":
                        nc.scalar.activation(
                            dst, u_b[:], _act_func(),
                            scale=tl[:, m : m + 1], alpha=NEG_SLOPE,
                        )
                    else:  # gbeta
                        y = op.tile([128, DOUT], F32, tag="y")
                        nc.vector.scalar_tensor_tensor(
                            out=y[:],
                            in0=u_b[:],
                            scalar=tl[:, m : m + 1],
                            in1=be_b,
                            op0=MULT,
                            op1=ADD,
                        )
                        nc.scalar.activation(dst, y[:], _act_func(), alpha=NEG_SLOPE)
                dma_eng = nc.sync if g % 2 == 0 else nc.scalar
                if _CFG["stores3d"]:
                    dma_eng.dma_start(
                        out[g * 256 : (g + 1) * 256, :].rearrange(
                            "(b p) f -> p b f", p=128
                        ),
                        ow.rearrange("p (b f) -> p b f", b=2),
                    )
                else:
                    for b in range(2):
                        m = 2 * g + b
                        dma_eng.dma_start(
                            out[m * 128 : (m + 1) * 128, :],
                            ow[:, b * DOUT : (b + 1) * DOUT],
                        )

    nc.compile()
    return nc


def _get_nc(variant: str):
    key = (variant, tuple(sorted(_CFG.items())))
    if key not in _CACHE:
        _CACHE[key] = _build(variant)
    return _CACHE[key]


def kernel(x, alphas_A, controls_A, controls_B, linear_bias, bn_gamma, bn_beta,
           _trace=False):
    x = np.asarray(x, dtype=np.float32)
    alphas_A = np.asarray(alphas_A, dtype=np.float32)
    controls_A = np.asarray(controls_A, dtype=np.float32)
    controls_B = np.asarray(controls_B, dtype=np.float32)
    bn_gamma = np.asarray(bn_gamma, dtype=np.float32)
    bn_beta = np.asarray(bn_beta, dtype=np.float32)

    import ml_dtypes

    A = (alphas_A * controls_A).sum(axis=0).astype(np.float32)          # (256,)
    Bv = (controls_B * alphas_A.T).sum(axis=1).astype(np.float32)       # (4096,)
    ga = (bn_gamma * A).astype(np.float32)
    cbv = np.ascontiguousarray(
        Bv.reshape(1, DIN).astype(ml_dtypes.bfloat16))
    cf = np.ascontiguousarray(
        np.concatenate([A * A, ga, bn_beta]).reshape(1, 3 * DOUT),
        dtype=np.float32)
    x_bf = np.ascontiguousarray(x.astype(ml_dtypes.bfloat16))

    with_beta = bool(np.any(bn_beta != 0))
    if with_beta:
        variant = "gbeta"
    elif bool(np.all(ga >= 0)):
        variant = "fast"
    else:
        variant = "gnb"

    nc = _get_nc(variant)
    in_maps = []
    for c in range(N_CORES):
        in_maps.append({
            "xs": np.ascontiguousarray(x_bf[c * B_SHARD : (c + 1) * B_SHARD]),
            "cbv": cbv,
            "cf": cf,
        })

    res = run_bass_kernel_spmd(
        nc, in_maps, core_ids=list(range(N_CORES)), trace=_trace
    )
    out = np.concatenate([r["out"] for r in res.results], axis=0)
    if _trace:
        return out, res
    return out


# revision 15
# speedup vs baseline: 1.5155x; 1.1134x over previous
"""Trainium2 Bass kernel for nn_MLPLoraSubspace.

Math: A = sum(alphas_A * controls_A, 0)  (256,)
      Bv = sum(alphas_A.T * controls_B, 1)  (4096,)
      W = A outer Bv  (rank-1)  -> out = (x @ Bv) outer A + bias
      BatchNorm(training stats) then LeakyReLU(0.2).

Because W is rank-1, out[i,j] = A[j]*t[i] + bias[j] with t = x @ Bv.
Batch stats:  mean_j = A_j*mean(t) + bias_j,  var_j = A_j^2*var(t), so
  act[i,j] = lrelu( u_j * (t[i]-mean_t) + beta_j ),
  u_j = gamma_j*A_j/sqrt(A_j^2*var_t+eps).  Bias cancels exactly.

Fast path (beta==0 and gamma*A >= 0, true for the reference inputs):
  lrelu(u_j * z) = u_j * lrelu(z), so the activation collapses onto the
  tiny t column and phase 3 is a pure outer product.

Sharding: data-parallel over batch, 8 cores x 2048 rows. Per-core partial
(sum t, sum t^2) is AllGather'd (2 floats) to form global batch stats.

v2 layout: x streamed as 16x 2MB tile DMAs alternating the two HWDGE
queues (sync/scalar); per tile ONE DVE tensor_tensor_reduce does
mult+rowsum; consts arrive as one [1,4864] strip and are broadcast
across partitions with TensorE matmuls against a ones vector (no 2MB
HBM broadcast); single collective warmup (two would outlast phase 1
and queue-block the real AllGather); epilogue is 8 wide outer-product
groups with stores alternating queues.
"""

import sys

for p in ("/opt/trn_rl_repo", "/root/.axon_site/_ro/trn_rl_repo"):
    if p not in sys.path:
        sys.path.insert(0, p)

import numpy as np

from concourse import bacc, bass, mybir, tile
from concourse.bass_utils import run_bass_kernel_spmd

F32 = mybir.dt.float32
BF16 = mybir.dt.bfloat16
ADD = mybir.AluOpType.add
MULT = mybir.AluOpType.mult
N_CORES = 8
B_FULL, DIN, DOUT = 16384, 4096, 256
B_SHARD = B_FULL // N_CORES          # 2048
M_TILES = B_SHARD // 128             # 16
CLEN = DIN + 3 * DOUT                # cvec: [Bv | A^2 | gamma*A | beta]
BN_EPS = 1e-5
NEG_SLOPE = 0.2

_CACHE = {}
# Bisection/config knobs (HW-debug): each key toggles one v2 feature.
# NOTE: tensor_tensor_reduce hangs TRN2 HW here (sim-only op, bisected
# 2026-08-09) -- phase 1/2 use DVE mult + ACT Copy-accum instead.
_CFG = {
    "stores3d": True, # paired 3D-view output stores (else per-tile 2D)
    "dualq": True,    # x loads alternate sync/scalar queues (else sync only)
    "warmups": 1,     # number of warmup collectives
    "nocc": False,    # timing probe: skip collectives, use local sums (WRONG output)
}
_ACT_FUNC_OVERRIDE = None    # for sim testing (sim lacks Prelu)
_SIM_RSQRT = False  # for sim testing (sim lacks Abs_reciprocal_sqrt)


def _act_func():
    return _ACT_FUNC_OVERRIDE or mybir.ActivationFunctionType.Prelu


def _emit_rsqrt(nc, out_ap, in_ap):
    if _SIM_RSQRT:
        nc.scalar.activation(out_ap, in_ap, mybir.ActivationFunctionType.Sqrt)
        nc.vector.reciprocal(out_ap, out_ap)
    else:
        nc.scalar.activation(
            out_ap, in_ap, mybir.ActivationFunctionType.Abs_reciprocal_sqrt
        )


def _build(variant: str):
    """variant: 'fast' (beta==0, u>=0), 'gnb' (beta==0), 'gbeta'."""
    nc = bacc.Bacc(
        "TRN2",
        target_bir_lowering=False,
        debug=False,
        enable_asserts=False,
        num_devices=N_CORES,
    )
    xs = nc.dram_tensor("xs", [B_SHARD, DIN], BF16, kind="ExternalInput").ap()
    cbv = nc.dram_tensor("cbv", [1, DIN], BF16, kind="ExternalInput").ap()
    cf = nc.dram_tensor("cf", [1, 3 * DOUT], F32, kind="ExternalInput").ap()
    out = nc.dram_tensor("out", [B_SHARD, DOUT], F32, kind="ExternalOutput").ap()

    with tile.TileContext(nc) as tc:
        with (
            tc.tile_pool(name="xp", bufs=8) as xp,
            tc.tile_pool(name="scr", bufs=3) as scrp,
            tc.tile_pool(name="scr3", bufs=1) as scr3p,
            tc.tile_pool(name="cst", bufs=1) as cst,
            tc.tile_pool(name="op", bufs=8) as op,
            tc.tile_pool(name="ps", bufs=1, space="PSUM") as ps,
            tc.tile_pool(name="dram", bufs=1, space="DRAM") as dram,
        ):
            # Warmup collective(s): absorb CC-stream/mesh first-call
            # setup (~75us) while phase 1 streams x. Result unused.
            for w in range(0 if _CFG["nocc"] else _CFG["warmups"]):
                wi = dram.tile([2, 1], F32, tag=f"wi{w}")
                wo = dram.tile([2 * N_CORES, 1], F32, tag=f"wo{w}")
                nc.gpsimd.collective_compute(
                    "AllGather",
                    mybir.AluOpType.bypass,
                    replica_groups=[list(range(N_CORES))],
                    ins=[wi.opt()],
                    outs=[wo.opt()],
                )

            # Consts: Bv (bf16) broadcast in two halves front-loaded on
            # BOTH HWDGE queues; the f32 [a2|ga|beta] strip behind the
            # scalar half.  All land by ~10us so phase-1 compute starts
            # immediately; x tiles queue right behind.
            bv_sb = cst.tile([128, DIN], BF16, tag="bv")
            nc.sync.dma_start(
                bv_sb[:], cbv[0:1, :].broadcast_to([128, DIN])
            )
            c_sb = cst.tile([128, 3 * DOUT], F32, tag="c")
            nc.scalar.dma_start(
                c_sb[:], cf[0:1, :].broadcast_to([128, 3 * DOUT])
            )

            ones1 = cst.tile([1, 128], F32, tag="ones1")
            nc.vector.memset(ones1[:], 1.0)
            ones128 = cst.tile([128, 1], F32, tag="ones128")
            nc.vector.memset(ones128[:], 1.0)

            # Warm the ACT table set needed at stats time as the FIRST ACT
            # op: the set also contains Copy (filler in every set), so ACT
            # does exactly one TABLE_LOAD for the whole kernel.
            dum = cst.tile([1, 1], F32, tag="dum")
            _emit_rsqrt(nc, dum[:], ones1[0:1, 0:1])
            if variant != "fast":
                dum2 = cst.tile([1, 1], F32, tag="dum2")
                nc.scalar.activation(
                    dum2[:], ones1[0:1, 0:1], _act_func(), alpha=NEG_SLOPE
                )

            a2_b = c_sb[:, 0:DOUT]
            ga_b = c_sb[:, DOUT : 2 * DOUT]
            be_b = c_sb[:, 2 * DOUT : 3 * DOUT]

            t_all = cst.tile([128, M_TILES], F32, tag="t")
            t_a = cst.tile([128, M_TILES], F32, tag="ta")
            t_b = cst.tile([128, M_TILES], F32, tag="tb")
            t_parts = cst.tile([128, 4], F32, tag="tparts")
            ASPLIT = 14 * 256  # ACT reduces cols [0:ASPLIT), DVE the rest

            # Phase 1: DVE mult + ACT Copy-accum per tile; x tiles
            # alternate the two HWDGE queues so per-DMA fixed costs
            # overlap and the SDMA engines never starve.  The last tile
            # is split into 4 column chunks so its mult+reduce pipelines
            # right behind the final DMA instead of adding ~8us serially.
            for m in range(M_TILES):
                x_sb = xp.tile([128, DIN], BF16, tag="x")
                eng = nc.sync if (m % 2 == 0 or not _CFG["dualq"]) else nc.scalar
                eng.dma_start(x_sb[:], xs[m * 128 : (m + 1) * 128, :])
                scr = scrp.tile([128, DIN], BF16, tag="scr")
                scr3 = scr3p.tile([128, DIN], BF16, tag="scr3")
                if m < M_TILES - 1:
                    nc.vector.tensor_mul(scr[:], x_sb[:], bv_sb[:])
                    nc.scalar.activation(
                        scr3[:, 0:ASPLIT],
                        scr[:, 0:ASPLIT],
                        mybir.ActivationFunctionType.Copy,
                        accum_out=t_a[:, m : m + 1],
                    )
                    nc.vector.tensor_reduce(
                        out=t_b[:, m : m + 1],
                        in_=scr[:, ASPLIT:DIN],
                        axis=mybir.AxisListType.X,
                        op=ADD,
                    )
                else:
                    q = DIN // 4
                    for c in range(4):
                        sl = slice(c * q, (c + 1) * q)
                        nc.vector.tensor_mul(scr[:, sl], x_sb[:, sl], bv_sb[:, sl])
                        if c < 3:
                            nc.scalar.activation(
                                scr3[:, sl],
                                scr[:, sl],
                                mybir.ActivationFunctionType.Copy,
                                accum_out=t_parts[:, c : c + 1],
                            )
                        else:
                            nc.vector.tensor_reduce(
                                out=t_parts[:, c : c + 1],
                                in_=scr[:, sl],
                                axis=mybir.AxisListType.X,
                                op=ADD,
                            )
                    nc.vector.tensor_add(
                        t_a[:, M_TILES - 1 : M_TILES],
                        t_parts[:, 0:1],
                        t_parts[:, 1:2],
                    )
                    nc.vector.tensor_add(
                        t_b[:, M_TILES - 1 : M_TILES],
                        t_parts[:, 2:3],
                        t_parts[:, 3:4],
                    )
            nc.vector.tensor_add(t_all[:], t_a[:], t_b[:])

            # Phase 2: local partial sums -> cross-partition matmul reduce
            # -> AllGather of [sum_t, sum_t2] per core.
            sp = cst.tile([128, 2], F32, tag="sp")
            nc.vector.tensor_reduce(
                out=sp[:, 0:1], in_=t_all[:], axis=mybir.AxisListType.X, op=ADD
            )
            t_cp = cst.tile([128, M_TILES], F32, tag="tcp")
            nc.vector.tensor_copy(t_cp[:], t_all[:])
            tsq = cst.tile([128, M_TILES], F32, tag="tsq")
            nc.vector.tensor_mul(tsq[:], t_all[:], t_cp[:])
            nc.vector.tensor_reduce(
                out=sp[:, 1:2], in_=tsq[:], axis=mybir.AxisListType.X, op=ADD
            )
            s_ps = ps.tile([2, 1], F32, tag="sps")
            nc.tensor.matmul(s_ps[:], sp[:], ones128[:], start=True, stop=True)
            s_sb = cst.tile([2, 1], F32, tag="ssb")
            nc.vector.tensor_copy(s_sb[:], s_ps[:])

            sb2 = cst.tile([128, 2], F32, tag="sb2")
            if _CFG["nocc"]:
                # timing probe only: pretend local sums are global
                nc.vector.tensor_scalar_mul(sb2[:], sp[:], float(N_CORES))
            else:
                bi = dram.tile([2, 1], F32, tag="bi")
                bo = dram.tile([2 * N_CORES, 1], F32, tag="bo")
                nc.sync.dma_start(bi[:], s_sb[:])
                nc.gpsimd.collective_compute(
                    "AllGather",
                    mybir.AluOpType.bypass,
                    replica_groups=[list(range(N_CORES))],
                    ins=[bi.opt()],
                    outs=[bo.opt()],
                )

                # Bring the 16 gathered floats back and replicate across
                # partitions with a tiny matmul (cheaper than a
                # DRE-broadcast DMA), then reduce over ranks with a
                # stride-2 view.
                s16 = cst.tile([1, 2 * N_CORES], F32, tag="s16")
                nc.sync.dma_start(s16[:], bo.rearrange("a b -> b a"))
                pb3 = ps.tile([128, 2 * N_CORES], F32, tag="p3")
                nc.tensor.matmul(
                    pb3[:], ones1[:], s16[:], start=True, stop=True
                )
                sb16 = cst.tile([128, 2 * N_CORES], F32, tag="sb16")
                nc.vector.tensor_copy(sb16[:], pb3[:])
                nc.vector.tensor_reduce(
                    out=sb2[:],
                    in_=sb16.rearrange("p (r s) -> p s r", s=2),
                    axis=mybir.AxisListType.X,
                    op=ADD,
                )

            # Stats math (replicated on all 128 partitions)
            mcol = cst.tile([128, 1], F32, tag="mcol")
            nc.vector.tensor_scalar_mul(mcol[:], sb2[:, 0:1], 1.0 / B_FULL)
            ecol = cst.tile([128, 1], F32, tag="ecol")
            nc.vector.tensor_scalar_mul(ecol[:], sb2[:, 1:2], 1.0 / B_FULL)
            msq = cst.tile([128, 1], F32, tag="msq")
            nc.vector.tensor_mul(msq[:], mcol[:], mcol[:])
            vcol = cst.tile([128, 1], F32, tag="vcol")
            nc.vector.tensor_sub(vcol[:], ecol[:], msq[:])

            v1 = cst.tile([128, DOUT], F32, tag="v1")
            nc.vector.tensor_scalar(
                v1[:], a2_b, vcol[:, 0:1], BN_EPS, op0=MULT, op1=ADD
            )
            v3 = cst.tile([128, DOUT], F32, tag="v3")
            _emit_rsqrt(nc, v3[:], v1[:])
            u_b = cst.tile([128, DOUT], F32, tag="ub")
            nc.vector.tensor_mul(u_b[:], v3[:], ga_b)

            tcall = cst.tile([128, M_TILES], F32, tag="tc")
            nc.vector.tensor_scalar_sub(tcall[:], t_all[:], mcol[:, 0:1])

            if variant == "fast":
                # lrelu commutes with the positive per-column scale u:
                # apply it to the tiny t column once, then phase 3 is a
                # pure outer product on DVE.
                tneg = cst.tile([128, M_TILES], F32, tag="tn")
                nc.vector.tensor_scalar_mul(tneg[:], tcall[:], NEG_SLOPE)
                tl = cst.tile([128, M_TILES], F32, tag="tl")
                nc.vector.tensor_tensor(
                    tl[:], tcall[:], tneg[:], op=mybir.AluOpType.max
                )
            else:
                tl = tcall

            # Phase 3: 4 groups of 4 row-tiles, stores alternate queues.
            for g in range(M_TILES // 4):
                ow = op.tile([128, 4 * DOUT], F32, tag="ow")
                for b in range(4):
                    m = 4 * g + b
                    dst = ow[:, b * DOUT : (b + 1) * DOUT]
                    if variant == "fast":
                        if m % 3 == 2:
                            # offload ~1/3 of the outer products to ACT
                            nc.scalar.activation(
                                dst, u_b[:],
                                mybir.ActivationFunctionType.Copy,
                                scale=tl[:, m : m + 1],
                            )
                        else:
                            nc.vector.tensor_scalar_mul(
                                dst, u_b[:], tl[:, m : m + 1]
                            )
                    elif variant == "gnb":
                        nc.scalar.activation(
                            dst, u_b[:], _act_func(),
                            scale=tl[:, m : m + 1], alpha=NEG_SLOPE,
                        )
                    else:  # gbeta
                        y = op.tile([128, DOUT], F32, tag="y")
                        nc.vector.scalar_tensor_tensor(
                            out=y[:],
                            in0=u_b[:],
                            scalar=tl[:, m : m + 1],
                            in1=be_b,
                            op0=MULT,
                            op1=ADD,
                        )
                        nc.scalar.activation(dst, y[:], _act_func(), alpha=NEG_SLOPE)
                dma_eng = nc.sync if g % 2 == 0 else nc.scalar
                if _CFG["stores3d"]:
                    dma_eng.dma_start(
                        out[g * 512 : (g + 1) * 512, :].rearrange(
                            "(b p) f -> p b f", p=128
                        ),
                        ow.rearrange("p (b f) -> p b f", b=4),
                    )
                else:
                    for b in range(4):
                        m = 4 * g + b
                        dma_eng.dma_start(
                            out[m * 128 : (m + 1) * 128, :],
                            ow[:, b * DOUT : (b + 1) * DOUT],
                        )

    nc.compile()
    return nc


def _get_nc(variant: str):
    key = (variant, tuple(sorted(_CFG.items())))
    if key not in _CACHE:
        _CACHE[key] = _build(variant)
    return _CACHE[key]


def kernel(x, alphas_A, controls_A, controls_B, linear_bias, bn_gamma, bn_beta,
           _trace=False):
    x = np.asarray(x, dtype=np.float32)
    alphas_A = np.asarray(alphas_A, dtype=np.float32)
    controls_A = np.asarray(controls_A, dtype=np.float32)
    controls_B = np.asarray(controls_B, dtype=np.float32)
    bn_gamma = np.asarray(bn_gamma, dtype=np.float32)
    bn_beta = np.asarray(bn_beta, dtype=np.float32)

    import ml_dtypes

    A = (alphas_A * controls_A).sum(axis=0).astype(np.float32)          # (256,)
    Bv = (controls_B * alphas_A.T).sum(axis=1).astype(np.float32)       # (4096,)
    ga = (bn_gamma * A).astype(np.float32)
    cbv = np.ascontiguousarray(
        Bv.reshape(1, DIN).astype(ml_dtypes.bfloat16))
    cf = np.ascontiguousarray(
        np.concatenate([A * A, ga, bn_beta]).reshape(1, 3 * DOUT),
        dtype=np.float32)
    x_bf = np.ascontiguousarray(x.astype(ml_dtypes.bfloat16))

    with_beta = bool(np.any(bn_beta != 0))
    if with_beta:
        variant = "gbeta"
    elif bool(np.all(ga >= 0)):
        variant = "fast"
    else:
        variant = "gnb"

    nc = _get_nc(variant)
    in_maps = []
    for c in range(N_CORES):
        in_maps.append({
            "xs": np.ascontiguousarray(x_bf[c * B_SHARD : (c + 1) * B_SHARD]),
            "cbv": cbv,
            "cf": cf,
        })

    res = run_bass_kernel_spmd(
        nc, in_maps, core_ids=list(range(N_CORES)), trace=_trace
    )
    out = np.concatenate([r["out"] for r in res.results], axis=0)
    if _trace:
        return out, res
    return out
